# revision 7
# baseline (speedup 1.0000x reference)
"""CapsuleLayer kernel for Trainium2, 8 NeuronCores.

Math: the reference's softmax is over a singleton axis, so c_ij == 1 and the
routing loop is dead code.  The output is exactly

    s[b, j, k]  = sum_{i, u} W[0, i, j, k, u] * x[b, u, i]
    m[b, k]     = sum_j s[b, j, k]^2
    v[b, j, k]  = (sqrt(m) / (1 + m)) * s[b, j, k]        (squash)

i.e. one (32 x 32768) @ (32768 x 1024) matmul plus a tiny per-(b,k)
epilogue.  W dominates: the kernel is HBM-bound on reading W once.

Sharding: the output column grid is (k, j) with k = unit_size (64).  Shard on
k: core c owns k in [8c, 8c+8).  Each core reads its W slice (read exactly
once machine-wide), the full x (replicated), and computes a fully local
squash (the j-reduction inside m is intact per core).  Zero cross-core
communication.

Numerics: operands are SINGLE bf16 (not hi/lo pairs).  Measured end metric
(max-abs-err / absmax) is ~4e-3 against the 2e-2 gate: the contraction error
is ~eps_bf16 relative and the squash output is scale-invariant in s.  This
halves HBM traffic vs an fp32-grade hi/lo scheme - per core ~8.4 MB W +
~2.1 MB x - and halves PE streaming time.

Each contraction k-tile is one matmul: stationary x [128, 32], moving W
[128, 128], PSUM [32, 128] fp32 accumulates across all 256 k-tiles.  The
epilogue reads PSUM directly (no fold needed).

Host-side, W is resliced into the exact SBUF streaming layout
[partition=contraction%128][ktile x column] so every DMA row is contiguous.
"""

import numpy as np

B, U, I, J, K = 32, 16, 2048, 16, 64  # batch, in_units, in_ch, num_units, unit_size
NC = 8                                # cores
KPC = K // NC                         # unit_size columns per core (8)
N = KPC * J                           # output columns per core (128), kk-major, j-minor
KK = I * U                            # contraction length (32768)
P = 128                               # partitions
KT = KK // P                          # contraction tiles (256)
# Chunk sizes (in contraction tiles): small first chunks so the PE starts
# right away instead of waiting on a big transfer; big middle chunks so DMA
# descriptors are 16 KB/partition-row (measured ~420 GB/s vs ~300 GB/s at
# 8 KB); small last chunks so the PE tail after the final DMA is short.
CHUNKS = [4, 8, 16, 32, 64, 64, 64, 2, 2]
assert sum(CHUNKS) == KT

_CACHE = {}

DEFAULT_CFG = dict(chunks=None, bufs=4, warm_sqrt=True, x_eng="scalar")


def _build(chunks=None, bufs=4, warm_sqrt=True, x_eng="scalar"):
    import concourse.bacc as bacc
    import concourse.tile as tile
    import concourse.mybir as mybir

    import concourse.bass as bass

    if chunks is None:
        chunks = CHUNKS
    assert sum(chunks) == KT

    f32 = mybir.dt.float32
    bf16 = mybir.dt.bfloat16
    nc = bacc.Bacc("TRN2", num_devices=NC, debug=False, enable_asserts=False)
    # x: per k-tile [128, 32] bf16 columns, k-tile-major
    x_d = nc.dram_tensor("x", (P, KT * B), bf16, kind="ExternalInput")
    # w: per k-tile [128, 128] bf16 columns, k-tile-major
    w_d = nc.dram_tensor("w", (P, KT * N), bf16, kind="ExternalInput")
    v_d = nc.dram_tensor("v", (B, KPC, J), f32, kind="ExternalOutput")

    maxch = max(chunks)
    with tile.TileContext(nc) as tc:
        with (
            tc.tile_pool(name="wp", bufs=bufs) as wp,
            tc.tile_pool(name="ep", bufs=1) as ep,
            tc.tile_pool(name="ps", bufs=1, space="PSUM") as ps,
        ):
            s_ps = ps.tile([B, KPC, J], f32)
            if warm_sqrt:
                # load the ACT sqrt table during the DMA phase, not in the
                # serial epilogue
                wtile = ep.tile([1, 1], f32)
                nc.gpsimd.memset(wtile[:], 1.0)
                nc.scalar.sqrt(wtile[:], wtile[:])
            x_dma = getattr(nc, x_eng).dma_start if x_eng else nc.sync.dma_start
            # x is small (2 MB): one DMA up front on its own issuing engine.
            # The PE streams faster than DMA steady-state, so its later start
            # (waiting on all of x) is absorbed before the W stream ends.
            x_sb = ep.tile([P, KT * B], bf16)
            x_dma(x_sb[:], x_d[:])
            kt0 = 0
            for ch in chunks:
                w_sb = wp.tile([P, maxch * N], bf16, tag="wch")
                nc.sync.dma_start(
                    w_sb[:, : ch * N],
                    w_d[:, kt0 * N : (kt0 + ch) * N],
                )
                for t in range(ch):
                    kt = kt0 + t
                    nc.tensor.matmul(
                        s_ps[:, :, :],
                        x_sb[:, kt * B : (kt + 1) * B],
                        w_sb[:, t * N : (t + 1) * N],
                        start=(kt == 0),
                        stop=(kt == KT - 1),
                    )
                kt0 += ch

            # epilogue: squash on [B, KPC, J]
            s_sb = ep.tile([B, KPC, J], f32)
            nc.vector.tensor_copy(s_sb[:], s_ps[:])
            s2 = ep.tile([B, KPC, J], f32)
            nc.vector.tensor_mul(s2[:], s_sb[:], s_sb[:])
            m = ep.tile([B, KPC], f32)
            nc.vector.reduce_sum(m[:], s2[:], axis=mybir.AxisListType.X)
            sq = ep.tile([B, KPC], f32)
            nc.scalar.sqrt(sq[:], m[:])
            d = ep.tile([B, KPC], f32)
            nc.vector.tensor_scalar_add(d[:], m[:], 1.0)
            r = ep.tile([B, KPC], f32)
            nc.vector.reciprocal(r[:], d[:])
            sc = ep.tile([B, KPC], f32)
            nc.vector.tensor_mul(sc[:], sq[:], r[:])
            v_sb = ep.tile([B, KPC, J], f32)
            sc_ap = sc[:]
            sc_bc = bass.AP(
                sc_ap.tensor,
                sc_ap.offset,
                [list(sc_ap.ap[0]), list(sc_ap.ap[1]), [0, J]],
            )
            nc.vector.tensor_mul(v_sb[:], s_sb[:], sc_bc)
            nc.sync.dma_start(v_d[:], v_sb[:])

    nc.compile()
    return nc


def get_nc(**cfg):
    key = ("nc", tuple(sorted((k, tuple(v) if isinstance(v, list) else v)
                              for k, v in cfg.items())))
    if key not in _CACHE:
        _CACHE[key] = _build(**cfg)
    return _CACHE[key]


def prep_inputs(x, W, cfg=None):
    """Full inputs -> per-core in_maps with the bf16 streaming layouts."""
    import ml_dtypes

    x = np.ascontiguousarray(np.asarray(x, dtype=np.float32))
    W = np.asarray(W, dtype=np.float32)
    assert x.shape == (B, U, I) and W.shape == (1, I, J, K, U)

    # x[b,u,i] -> [KK=(i major, u minor), b] -> bf16 [P, KT*B]
    xm = x.transpose(2, 1, 0).reshape(KT, P, B).astype(ml_dtypes.bfloat16)
    xhost = np.ascontiguousarray(xm.transpose(1, 0, 2).reshape(P, KT * B))

    in_maps = []
    W0 = W[0]  # [I, J, K, U]
    for c in range(NC):
        Wc = W0[:, :, c * KPC : (c + 1) * KPC, :]          # [I, J, KPC, U]
        wm = (
            Wc.transpose(0, 3, 2, 1)
            .reshape(KT, P, N)
            .astype(ml_dtypes.bfloat16)
        )
        whost = np.ascontiguousarray(wm.transpose(1, 0, 2).reshape(P, KT * N))
        in_maps.append({"x": xhost, "w": whost})
    return in_maps


def gather_output(results):
    """Per-core "v" [B, KPC, J] -> full [B, J, K]."""
    out = np.empty((B, J, K), dtype=np.float32)
    for c in range(NC):
        out[:, :, c * KPC : (c + 1) * KPC] = results[c]["v"].transpose(0, 2, 1)
    return out


def run(x, W, cfg=None, in_maps=None, **spmd_kwargs):
    from concourse import bass_utils

    if cfg is None:
        cfg = DEFAULT_CFG
    nc = get_nc(**cfg)
    if in_maps is None:
        in_maps = prep_inputs(x, W, cfg=cfg)
    res = bass_utils.run_bass_kernel_spmd(
        nc, in_maps, core_ids=list(range(NC)), **spmd_kwargs
    )
    return gather_output(res.results), res


def kernel(x, W):
    out, _ = run(x, W)
    return out
